# revision 1
# baseline (speedup 1.0000x reference)
"""Trainium2 Bass kernel for CRFSegmentationModel (conv backbone + CRF Viterbi).

Sharding: batch 16 -> 8 cores x 2 samples each (pure data parallelism).

Per-core:
  conv1(3x3 SAME 3->256)+relu and conv2(1x1 256->21) as PE matmuls; emissions
  stored to HBM in [t, n] layout (t = y*128+x).

  The L=16384-step Viterbi scan is parallelized over K=64 chunks x 2 samples
  = 128 chains (one SBUF partition each). Chunks warm up W=128 steps from a
  constant magnitude-matched init (magnitudes from pass-1 zero-init gain
  probes); running at the reference's fp32 magnitude reproduces its argmax
  decisions (incl. rounding-collapsed ties) bit-exactly.

  Backpointers via packed key (tmp - M)*2^38 - p (first-index tie-break).
  Pass-3 walks chunks backward for all 21 candidate boundary tags (one-hot
  compose); a K-step threading pass then picks the true boundary tags and a
  final masked reduction extracts the tag sequence.
"""
import numpy as np

import concourse.bacc as bacc
import concourse.mybir as mybir
from concourse.bass_types import AP
from concourse.tile import TileContext
from concourse import bass_utils

F32 = mybir.dt.float32
AT = mybir.AluOpType
AX = mybir.AxisListType

B, C_IN, H, W_IMG = 16, 3, 128, 128
HID, C = 256, 21
L = H * W_IMG
NCORES = 8
BL = B // NCORES

K = 64            # chunks per sample
S = L // K        # 256
WU = 32           # pass-2 warmup
P1W = 16          # pass-1 warmup
P1G = 32          # pass-1 gain span
CH = BL * K       # 128 chains
F = C * C
BIG = float(2.0 ** 38)
EMPAD = WU - 1                    # rows for t<0
EMLEN = EMPAD + L + S + 1
ULEN = WU + S                     # em steps per chain

_CACHE = {}
LAST_EXEC_NS = None


def _register_dve_ops():
    """Runtime-register two fused DVE ops (idempotent, self-contained)."""
    import concourse.dve_ops as D
    from concourse.dve_spec import (Spec, Src0, Src1, C0, C1, Zero, select, eq,
                                    Idx, SubIdx, lower, _has_src1)
    from concourse.dve_uop import DveOpSpec
    from concourse.dve_table_gen import dve_ver_for
    if "ANT_DKEY" in D._SUB_OPCODE_FOR_NAME:
        return {o.name: o for o in D.OPS}

    def dkey_ref(in0, in1, c0, c1, c2):
        jj = np.arange(in0.shape[2], dtype=np.float32)[None, None, :]
        return ((in0.astype(np.float32) - in1.astype(np.float32))
                * np.float32(c1) - jj).astype(np.float32)

    def selmul_ref(in0, in1, c0, c1, c2):
        jj = np.arange(in0.shape[2], dtype=np.float32)[None, None, :]
        return np.where(in0 == -jj, in1, np.float32(0.0)).astype(np.float32)

    jterm = Idx - SubIdx * C0
    specs = [
        ("ANT_DKEY", Spec(body=(Src0 - Src1) * C1 - jterm, reference=dkey_ref)),
        ("ANT_SELMUL", Spec(body=select(eq(Src0, Zero - jterm), Src1, Zero),
                            reference=selmul_ref)),
    ]
    ver = dve_ver_for("TRN2")
    for name, spec in specs:
        opcode = max(D._SUB_OPCODE_FOR_NAME.values()) + 1
        D._SUB_OPCODE_FOR_NAME[name] = opcode
        compiled = DveOpSpec(name=name, opcode=opcode, uops=lower(spec, ver=ver),
                             rd1_en=_has_src1(spec))
        op = D.DveOp(name, spec, subdim=True, uops_sha={ver: compiled.sha(ver)})
        D._COMPILE_CACHE[(name, ver)] = compiled
        D.OPS.append(op)
        D.CUSTOM_DVE_SPECS[name] = spec
    assert max(D._SUB_OPCODE_FOR_NAME.values()) < 0x20
    return {o.name: o for o in D.OPS}


def _r3(ap, inner=C):
    return ap.rearrange("p (a b) -> p a b", b=inner)


def _build():
    if "nc" in _CACHE:
        return _CACHE["nc"]
    ops = _register_dve_ops()
    DKEY, SELMUL = ops["ANT_DKEY"], ops["ANT_SELMUL"]
    nc = bacc.Bacc("TRN2", target_bir_lowering=False, debug=False, num_devices=1)

    x_d = nc.dram_tensor("x", (BL, C_IN, H, W_IMG), F32, kind="ExternalInput").ap()
    w1_d = nc.dram_tensor("w1i", (27, HID), F32, kind="ExternalInput").ap()
    b1_d = nc.dram_tensor("b1", (128, 2), F32, kind="ExternalInput").ap()
    w2_d = nc.dram_tensor("w2e", (128, 2 * C), F32, kind="ExternalInput").ap()
    b2_d = nc.dram_tensor("b2", (1, C), F32, kind="ExternalInput").ap()
    startrep_d = nc.dram_tensor("start_rep", (BL, C), F32, kind="ExternalInput").ap()
    endrep_d = nc.dram_tensor("end_rep", (BL, C), F32, kind="ExternalInput").ap()
    transrep_d = nc.dram_tensor("transrep", (CH, F), F32, kind="ExternalInput").ap()
    iotarep_d = nc.dram_tensor("iotarep", (CH, F), F32, kind="ExternalInput").ap()
    iotaneg_d = nc.dram_tensor("iotaneg", (CH, F), F32, kind="ExternalInput").ap()
    ident_d = nc.dram_tensor("identoh", (CH, F), F32, kind="ExternalInput").ap()
    negi21b_d = nc.dram_tensor("negi21b", (BL, C), F32, kind="ExternalInput").ap()
    zeros_d = nc.dram_tensor("zrow", (1, (S + 1) * C), F32, kind="ExternalInput").ap()

    tags_d = nc.dram_tensor("tags", (BL, L), F32, kind="ExternalOutput").ap()

    em_d = nc.dram_tensor("em_hbm", (BL, EMLEN, C), F32, kind="Internal").ap()
    bounce_d = nc.dram_tensor("bounce", (BL, K * C), F32, kind="Internal").ap()
    gsum_d = nc.dram_tensor("gsum", (2, CH), F32, kind="Internal").ap()

    em_flat = [em_d[b].rearrange("t n -> (t n)") for b in range(BL)]

    with TileContext(nc) as tc:
        # ====================== conv backbone ======================
        with tc.tile_pool(name="convpool", bufs=1) as cp, \
             tc.tile_pool(name="convwork", bufs=3) as cw, \
             tc.tile_pool(name="convpsum", bufs=2, space="PSUM") as cpp:
            xpad = cp.tile([C_IN, 130 * 130], F32)
            im2col = cp.tile([27, L], F32)
            w1sb = cp.tile([27, HID], F32)
            b1sb = cp.tile([128, 2], F32)
            w2sb = cp.tile([128, 2 * C], F32)
            b2sb = cp.tile([1, C], F32)
            onesb = cp.tile([1, 128], F32)
            zrow = cp.tile([1, (S + 1) * C], F32)

            nc.sync.dma_start(w1sb[:], w1_d[:])
            nc.sync.dma_start(b1sb[:], b1_d[:])
            nc.sync.dma_start(w2sb[:], w2_d[:])
            nc.sync.dma_start(b2sb[:], b2_d[:])
            nc.sync.dma_start(zrow[:], zeros_d[:])
            nc.vector.memset(onesb[:], 1.0)

            for b in range(BL):
                nc.vector.memset(xpad[:], 0.0)
                nc.sync.dma_start(
                    xpad[:].rearrange("p (y xx) -> p y xx", xx=130)[:, 1:129, 1:129],
                    x_d[b],
                )
                for dy in range(3):
                    for dx in range(3):
                        r0 = (dy * 3 + dx) * 3
                        nc.sync.dma_start(
                            im2col[r0:r0 + 3, :].rearrange("p (y xx) -> p y xx", xx=128),
                            xpad[:].rearrange("p (y xx) -> p y xx", xx=130)[
                                :, dy:dy + 128, dx:dx + 128],
                        )
                # front/tail zero pads of em
                nc.sync.dma_start(
                    AP(tensor=em_flat[b].tensor, offset=b * EMLEN * C,
                       ap=[[0, 1], [1, EMPAD * C]]),
                    zrow[:, 0:EMPAD * C])
                nc.sync.dma_start(
                    AP(tensor=em_flat[b].tensor,
                       offset=b * EMLEN * C + (EMPAD + L) * C,
                       ap=[[0, 1], [1, (S + 1) * C]]),
                    zrow[:])

                for tt in range(0, L, 512):
                    hid0 = cw.tile([128, 512], F32, tag="hid0")
                    hid1 = cw.tile([128, 512], F32, tag="hid1")
                    for hti, hid in ((0, hid0), (1, hid1)):
                        ps = cpp.tile([128, 512], F32, tag="psc1")
                        nc.tensor.matmul(
                            ps[:], w1sb[:, hti * 128:(hti + 1) * 128],
                            im2col[:, tt:tt + 512], start=True, stop=True)
                        nc.scalar.activation(
                            hid[:], ps[:], mybir.ActivationFunctionType.Relu,
                            bias=b1sb[:, hti:hti + 1], scale=1.0)
                    ps2 = cpp.tile([128, 4 * C], F32, tag="psc2")
                    for blk in range(4):
                        t0 = blk * 128
                        o = ps2[:, blk * C:(blk + 1) * C]
                        nc.tensor.matmul(o, hid0[:, t0:t0 + 128], w2sb[:, 0:C],
                                         start=True, stop=False)
                        nc.tensor.matmul(o, hid1[:, t0:t0 + 128], w2sb[:, C:2 * C],
                                         start=False, stop=False)
                        nc.tensor.matmul(o, onesb[:], b2sb[:], start=False, stop=True)
                    emst = cw.tile([128, 4 * C], F32, tag="emst")
                    nc.scalar.activation(
                        emst[:], ps2[:], mybir.ActivationFunctionType.Copy, scale=1.0)
                    nc.sync.dma_start(
                        em_d[b, EMPAD + tt:EMPAD + tt + 512]
                        .rearrange("(blk p) n -> p blk n", p=128),
                        _r3(emst[:], C).rearrange("p (blk n) -> p blk n", n=C)
                        if False else emst[:].rearrange("p (blk n) -> p blk n", n=C),
                    )

        # ====================== viterbi ======================
        with tc.tile_pool(name="vit", bufs=1) as vp:
            emt = vp.tile([CH, ULEN * C], F32)
            transrep = vp.tile([CH, F], F32)
            score = vp.tile([CH, C], F32)
            m_work = vp.tile([CH, C], F32)
            tmp_ring = vp.tile([CH, 8 * F], F32)
            m_ring = vp.tile([CH, 8 * C], F32)
            key_batch = vp.tile([CH, 8 * F], F32)
            hist = vp.tile([CH, S * C], F32)
            paths = vp.tile([CH, S * C], F32)
            msum0 = vp.tile([CH, 1], F32)
            msum1 = vp.tile([CH, 1], F32)
            gp = vp.tile([1, 3 * CH], F32)
            vinit = vp.tile([CH, 1], F32)
            s0t = vp.tile([BL, C], F32)
            fs = vp.tile([BL, C], F32)
            startrep = vp.tile([BL, C], F32)
            endrep = vp.tile([BL, C], F32)
            negi21b = vp.tile([BL, C], F32)
            ltoh = vp.tile([BL, C], F32)
            ohc = vp.tile([BL, K * C], F32)
            fmap = vp.tile([BL, K * C], F32)
            selmask = vp.tile([CH, C], F32)
            ohw = vp.tile([BL, C], F32)
            small = vp.tile([BL, C], F32)
            small1 = vp.tile([BL, 1], F32)
            tagsf = vp.tile([CH, S], F32)
            prod = vp.tile([CH, F], F32)
            selp = vp.tile([CH, S * C], F32)

            nc.sync.dma_start(transrep[:], transrep_d[:])
            nc.sync.dma_start(startrep[:], startrep_d[:])
            nc.sync.dma_start(endrep[:], endrep_d[:])
            nc.sync.dma_start(negi21b[:], negi21b_d[:])

            # em chunk rows: chain (b,c) covers t in [cS-WU+1, cS+S]
            # em index (EMPAD + t)*C ; row offset = b*EMLEN*C + c*S*C
            for b in range(BL):
                nc.sync.dma_start(
                    emt[b * K:(b + 1) * K, :],
                    AP(tensor=em_flat[b].tensor, offset=b * EMLEN * C,
                       ap=[[S * C, K], [1, ULEN * C]]))

            # score0 = em[t=0] + start
            nc.sync.dma_start(s0t[:], em_d[:, EMPAD, :])
            nc.vector.tensor_tensor(out=s0t[:], in0=s0t[:], in1=startrep[:], op=AT.add)

            def emsl(u):
                return emt[:, u * C:(u + 1) * C]

            def step(u_em, m_dst):
                nc.vector.tensor_tensor(
                    out=_r3(tmp_cur), in0=score[:].unsqueeze(1).broadcast_to((CH, C, C)),
                    in1=_r3(transrep[:]), op=AT.add)
                nc.vector.tensor_reduce(out=m_dst, in_=_r3(tmp_cur), axis=AX.X, op=AT.max)
                nc.vector.tensor_tensor(out=score[:], in0=m_dst, in1=emsl(u_em), op=AT.add)

            # ---------- pass 1 ----------
            nc.vector.memset(score[:], 0.0)
            tmp_cur = tmp_ring[:, 0:F]
            for s in range(P1W + P1G):
                step(WU - P1W + s, m_work[:])
                if s == P1W - 1:
                    nc.vector.tensor_reduce(out=msum0[:], in_=score[:], axis=AX.X, op=AT.add)
            nc.vector.tensor_reduce(out=msum1[:], in_=score[:], axis=AX.X, op=AT.add)

            nc.sync.dma_start(AP(tensor=gsum_d.tensor, offset=0, ap=[[1, CH], [1, 1]]), msum0[:])
            nc.sync.dma_start(AP(tensor=gsum_d.tensor, offset=CH, ap=[[1, CH], [1, 1]]), msum1[:])
            nc.sync.dma_start(gp[0:1, 0:2 * CH], gsum_d[:].rearrange("a b -> (a b)").unsqueeze(0))
            # g*S per chain -> gp[0, 2CH:3CH]
            nc.vector.tensor_tensor(out=gp[0:1, 2 * CH:3 * CH], in0=gp[0:1, CH:2 * CH],
                                    in1=gp[0:1, 0:CH], op=AT.subtract)
            nc.vector.tensor_scalar(out=gp[0:1, 2 * CH:3 * CH], in0=gp[0:1, 2 * CH:3 * CH],
                                    scalar1=float(S) / (C * P1G), scalar2=None, op0=AT.mult)
            # exclusive prefix into gp[0, 0:CH] (ping-pong to avoid in-place hazard)
            pfa = vp.tile([1, CH], F32)
            pfb = vp.tile([1, CH], F32)
            nc.vector.memset(pfa[:], 0.0)
            nc.vector.tensor_copy(pfa[0:1, 1:K], gp[0:1, 2 * CH:2 * CH + K - 1])
            nc.vector.tensor_copy(pfa[0:1, K + 1:2 * K], gp[0:1, 2 * CH + K:2 * CH + 2 * K - 1])
            cur, nxt = pfa, pfb
            for sh in (1, 2, 4, 8, 16, 32):
                for h0 in (0, K):
                    nc.vector.tensor_copy(nxt[0:1, h0:h0 + sh], cur[0:1, h0:h0 + sh])
                    nc.vector.tensor_tensor(
                        out=nxt[0:1, h0 + sh:h0 + K], in0=cur[0:1, h0 + sh:h0 + K],
                        in1=cur[0:1, h0:h0 + K - sh], op=AT.add)
                cur, nxt = nxt, cur
            nc.vector.tensor_copy(gp[0:1, 0:CH], cur[0:1, 0:CH])
            # per-sample base mean(score0)/C
            nc.vector.tensor_reduce(out=small1[:], in_=s0t[:], axis=AX.X, op=AT.add)
            nc.vector.tensor_scalar(out=small1[:], in0=small1[:], scalar1=1.0 / C,
                                    scalar2=None, op0=AT.mult)
            nc.sync.dma_start(bounce_d[0:1, 0:1], small1[0:1, :])
            nc.sync.dma_start(bounce_d[0:1, 1:2], small1[1:2, :])
            base2 = vp.tile([1, 2], F32)
            nc.sync.dma_start(base2[:], bounce_d[0:1, 0:2])
            nc.vector.tensor_scalar(out=gp[0:1, 0:K], in0=gp[0:1, 0:K],
                                    scalar1=base2[0:1, 0:1], scalar2=None, op0=AT.add)
            nc.vector.tensor_scalar(out=gp[0:1, K:2 * K], in0=gp[0:1, K:2 * K],
                                    scalar1=base2[0:1, 1:2], scalar2=None, op0=AT.add)
            nc.sync.dma_start(gsum_d[0:1, :], gp[0:1, 0:CH])
            nc.sync.dma_start(vinit[:], AP(tensor=gsum_d.tensor, offset=0, ap=[[1, CH], [1, 1]]))

            # ---------- pass 2 ----------
            nc.vector.memset(score[:], 0.0)
            nc.vector.tensor_scalar(out=score[:], in0=score[:], scalar1=vinit[:, :],
                                    scalar2=None, op0=AT.add)
            for s in range(WU + S):
                if s == WU:
                    # chunk 0 records from the exact t=0 state
                    nc.sync.dma_start(score[0:1, :], s0t[0:1, :])
                    nc.sync.dma_start(score[K:K + 1, :], s0t[1:2, :])
                rec = s >= WU
                r = s - WU
                slot = (r % 8) if rec else 7
                tmp_cur = tmp_ring[:, slot * F:(slot + 1) * F]
                m_dst = m_ring[:, slot * C:(slot + 1) * C] if rec else m_work[:]
                step(s, m_dst)
                if s == WU + S - 2:
                    nc.sync.dma_start(fs[0:1, :], score[K - 1:K, :])
                    nc.sync.dma_start(fs[1:2, :], score[CH - 1:CH, :])
                if rec and (r % 8 == 7):
                    r0 = r - 7
                    nc.vector._custom_dve(
                        DKEY,
                        out=key_batch[:].rearrange("p (sn q) -> p sn q", q=C),
                        in0=tmp_ring[:].rearrange("p (sn q) -> p sn q", q=C),
                        in1=m_ring[:].unsqueeze(2).broadcast_to((CH, 8 * C, C)),
                        s0=float(C), s1=BIG)
                    nc.vector.tensor_reduce(
                        out=hist[:, r0 * C:(r0 + 8) * C],
                        in_=key_batch[:].rearrange("p (sn q) -> p sn q", q=C),
                        axis=AX.X, op=AT.max)

            # identity-fix hist row S-1 of last chain of each sample
            nc.sync.dma_start(hist[K - 1:K, (S - 1) * C:], negi21b_d[0:1, :])
            nc.sync.dma_start(hist[CH - 1:CH, (S - 1) * C:], negi21b_d[0:1, :])

            # last tag onehot
            nc.vector.tensor_tensor(out=fs[:], in0=fs[:], in1=endrep[:], op=AT.add)
            nc.vector.tensor_reduce(out=small1[:], in_=fs[:], axis=AX.X, op=AT.max)
            nc.vector.tensor_scalar(out=small[:], in0=fs[:], scalar1=small1[:, :],
                                    scalar2=BIG, op0=AT.subtract, op1=AT.mult)
            nc.vector.tensor_tensor(out=small[:], in0=small[:], in1=negi21b[:], op=AT.add)
            nc.vector.tensor_reduce(out=small1[:], in_=small[:], axis=AX.X, op=AT.max)
            nc.vector.tensor_scalar(out=ltoh[:], in0=small[:], scalar1=small1[:, :],
                                    scalar2=None, op0=AT.is_equal)

            # ---------- pass 3 (fused select-eq-mul custom op) ----------
            idprev = vp.tile([CH, C], F32)
            nc.sync.dma_start(idprev[:],
                              AP(tensor=negi21b_d.tensor, offset=0, ap=[[0, CH], [1, C]]))
            for r in range(S - 1, -1, -1):
                hrow = hist[:, r * C:(r + 1) * C]
                prv = idprev[:] if r == S - 1 else paths[:, (r + 1) * C:(r + 2) * C]
                nc.vector._custom_dve(
                    SELMUL, out=_r3(prod[:]),
                    in0=prv.unsqueeze(2).broadcast_to((CH, C, C)),
                    in1=hrow.unsqueeze(1).broadcast_to((CH, C, C)), s0=float(C))
                nc.vector.tensor_reduce(
                    out=paths[:, r * C:(r + 1) * C], in_=_r3(prod[:]), axis=AX.X, op=AT.add)

            # ---------- threading ----------
            for b in range(BL):
                nc.sync.dma_start(
                    AP(tensor=bounce_d.tensor, offset=b * K * C, ap=[[C, K], [1, C]]),
                    paths[b * K:(b + 1) * K, 0:C])
            nc.sync.dma_start(fmap[:], bounce_d[:])
            nc.vector.tensor_copy(ohw[:], ltoh[:])
            for c in range(K - 1, -1, -1):
                nc.vector.tensor_copy(ohc[:, c * C:(c + 1) * C], ohw[:])
                if c > 0:
                    nc.vector.tensor_tensor(out=small[:], in0=ohw[:],
                                            in1=fmap[:, c * C:(c + 1) * C], op=AT.mult)
                    nc.vector.tensor_reduce(out=small1[:], in_=small[:], axis=AX.X, op=AT.add)
                    nc.vector.tensor_scalar(out=ohw[:], in0=negi21b[:],
                                            scalar1=small1[:, :], scalar2=None,
                                            op0=AT.is_equal)
            nc.sync.dma_start(bounce_d[:], ohc[:])
            for b in range(BL):
                nc.sync.dma_start(
                    selmask[b * K:(b + 1) * K, :],
                    AP(tensor=bounce_d.tensor, offset=b * K * C, ap=[[C, K], [1, C]]))

            # ---------- selection + output ----------
            nc.vector.tensor_tensor(
                out=selp[:].rearrange("p (r e) -> p r e", e=C),
                in0=paths[:].rearrange("p (r e) -> p r e", e=C),
                in1=selmask[:].unsqueeze(1).broadcast_to((CH, S, C)), op=AT.mult)
            nc.vector.tensor_reduce(
                out=tagsf[:], in_=selp[:].rearrange("p (r e) -> p r e", e=C),
                axis=AX.X, op=AT.add)
            nc.vector.tensor_scalar(out=tagsf[:], in0=tagsf[:], scalar1=-1.0,
                                    scalar2=None, op0=AT.mult)
            for b in range(BL):
                nc.sync.dma_start(
                    tags_d[b].rearrange("(c r) -> c r", r=S),
                    tagsf[b * K:(b + 1) * K, :])

    nc.compile()
    _CACHE["nc"] = nc
    return nc


def _consts():
    if "consts" not in _CACHE:
        iotarep = np.tile(np.tile(np.arange(C, dtype=np.float32), C)[None, :], (CH, 1))
        iotaneg = np.tile(np.tile(-np.arange(C, dtype=np.float32), C)[None, :], (CH, 1))
        ident = np.tile(np.eye(C, dtype=np.float32).reshape(1, F), (CH, 1))
        negi21b = np.tile(-np.arange(C, dtype=np.float32)[None, :], (BL, 1))
        zrow = np.zeros((1, (S + 1) * C), np.float32)
        _CACHE["consts"] = (iotarep, iotaneg, ident, negi21b, zrow)
    return _CACHE["consts"]


def kernel(x, conv1_w, conv1_b, conv2_w, conv2_b, start_trans, end_trans, trans):
    x = np.ascontiguousarray(np.asarray(x, np.float32))
    nc = _build()
    iotarep, iotaneg, ident, negi21b, zrow = _consts()

    trans = np.asarray(trans, np.float32)
    transrep = np.tile(np.ascontiguousarray(trans.T).reshape(1, F), (CH, 1)).astype(np.float32)
    w1i = np.ascontiguousarray(
        np.asarray(conv1_w, np.float32).transpose(2, 3, 1, 0).reshape(27, HID))
    b1 = np.ascontiguousarray(np.asarray(conv1_b, np.float32).reshape(2, 128).T)
    w2e = np.ascontiguousarray(np.asarray(conv2_w, np.float32).reshape(C, HID).T.reshape(2, 128, C).transpose(1, 0, 2).reshape(128, 2 * C))
    b2 = np.asarray(conv2_b, np.float32).reshape(1, C)
    startrep = np.tile(np.asarray(start_trans, np.float32).reshape(1, C), (BL, 1))
    endrep = np.tile(np.asarray(end_trans, np.float32).reshape(1, C), (BL, 1))

    in_maps = []
    for core in range(NCORES):
        in_maps.append({
            "x": np.ascontiguousarray(x[core * BL:(core + 1) * BL]),
            "w1i": w1i, "b1": b1, "w2e": w2e, "b2": b2,
            "start_rep": startrep, "end_rep": endrep,
            "transrep": transrep, "iotarep": iotarep, "iotaneg": iotaneg,
            "identoh": ident, "negi21b": negi21b, "zrow": zrow,
        })
    import os
    trace = bool(os.environ.get("BASS_TRACE_RUN"))
    res = bass_utils.run_bass_kernel_spmd(nc, in_maps, core_ids=list(range(NCORES)),
                                          trace=trace)
    global LAST_EXEC_NS
    LAST_EXEC_NS = res.exec_time_ns
    out = np.concatenate([r["tags"] for r in res.results], axis=0)
    return np.rint(out).astype(np.int32).reshape(B, H, W_IMG)



# revision 55
# speedup vs baseline: 195.8263x; 195.8263x over previous
"""Trainium2 Bass kernel for CRFSegmentationModel (conv backbone + CRF Viterbi).

Sharding: batch 16 -> 8 cores x 2 samples each (pure data parallelism).

Per-core:
  conv1(3x3 SAME 3->256)+relu and conv2(1x1 256->21) as PE matmuls; emissions
  stored to HBM in [t, n] layout (t = y*128+x).

  The L=16384-step Viterbi scan is parallelized over K=64 chunks x 2 samples
  = 128 chains (one SBUF partition each). Chunks warm up WU=32 steps from a
  zero init (max-plus recursion is shift-invariant, so the init offset only
  perturbs fp rounding, not decisions, once the warmup has coupled).

  Backpointers via packed key (tmp - M)*2^38 - p (first-index tie-break).
  Pass-3 walks chunks backward for all 21 candidate boundary tags (one-hot
  compose); a K-step threading pass then picks the true boundary tags and a
  final masked reduction extracts the tag sequence.
"""
import numpy as np

import concourse.bacc as bacc
import concourse.mybir as mybir
from concourse.bass_types import AP
from concourse.tile import TileContext
from concourse import bass_utils

F32 = mybir.dt.float32
I8 = mybir.dt.int8
AT = mybir.AluOpType
AX = mybir.AxisListType

B, C_IN, H, W_IMG = 16, 3, 128, 128
HID, C = 256, 21
L = H * W_IMG
NCORES = 8
BL = B // NCORES

K = 64            # chunks per sample
S = L // K        # 256
WU = 32           # warmup steps per chunk
P1W = 16          # pass-1 warmup
P1G = 32          # pass-1 gain span
CH = BL * K       # 128 chains
F = C * C
BIG = float(2.0 ** 38)
EMPAD = WU - 1                    # rows for t<0
EMLEN = EMPAD + L + S + 1
ULEN = WU + S                     # em steps per chain

_CACHE = {}
LAST_EXEC_NS = None


def _register_dve_ops():
    """Runtime-register two fused DVE ops (idempotent, self-contained)."""
    import concourse.dve_ops as D
    from concourse.dve_spec import (Spec, Src0, Src1, C0, C1, Zero, select, eq,
                                    Idx, SubIdx, lower, _has_src1)
    from concourse.dve_uop import DveOpSpec
    from concourse.dve_table_gen import dve_ver_for
    if "ANT_DKEY" in D._SUB_OPCODE_FOR_NAME:
        return {o.name: o for o in D.OPS}

    def dkey_ref(in0, in1, c0, c1, c2):
        jj = np.arange(in0.shape[2], dtype=np.float32)[None, None, :]
        return ((in0.astype(np.float32) - in1.astype(np.float32))
                * np.float32(c1) - jj).astype(np.float32)

    def selmul_ref(in0, in1, c0, c1, c2):
        jj = np.arange(in0.shape[2], dtype=np.float32)[None, None, :]
        return np.where(in0 == -jj, in1, np.float32(0.0)).astype(np.float32)

    def selmax_ref(in0, in1, c0, c1, c2):
        jj = np.arange(in0.shape[2], dtype=np.float32)[None, None, :]
        return np.where(in0 == -jj, in1, np.float32(c1)).astype(np.float32)

    jterm = Idx - SubIdx * C0
    specs = [
        ("ANT_DKEY", Spec(body=(Src0 - Src1) * C1 - jterm, reference=dkey_ref)),
        ("ANT_SELMUL", Spec(body=select(eq(Src0, Zero - jterm), Src1, Zero),
                            reference=selmul_ref)),
        ("ANT_SELMAX", Spec(body=select(eq(Src0, Zero - jterm), Src1, C1),
                            reference=selmax_ref)),
    ]
    ver = dve_ver_for("TRN2")
    for name, spec in specs:
        opcode = max(D._SUB_OPCODE_FOR_NAME.values()) + 1
        D._SUB_OPCODE_FOR_NAME[name] = opcode
        compiled = DveOpSpec(name=name, opcode=opcode, uops=lower(spec, ver=ver),
                             rd1_en=_has_src1(spec))
        op = D.DveOp(name, spec, subdim=True, uops_sha={ver: compiled.sha(ver)})
        D._COMPILE_CACHE[(name, ver)] = compiled
        D.OPS.append(op)
        D.CUSTOM_DVE_SPECS[name] = spec
    assert max(D._SUB_OPCODE_FOR_NAME.values()) < 0x20
    return {o.name: o for o in D.OPS}


def _r3(ap, inner=C):
    return ap.rearrange("p (a b) -> p a b", b=inner)


def _sum_tree(eng, gview):
    """In-place 21-group sums into position 0 of each group.

    gview: [P, G, 21]; exact for 0/1 or one-hot-masked data (order-free)."""
    for (d0, s0, n) in ((0, 10, 10), (0, 5, 5), (0, 2, 2), (0, 1, 1),
                        (0, 4, 1), (0, 20, 1)):
        eng.tensor_tensor(out=gview[:, :, d0:d0 + n],
                          in0=gview[:, :, d0:d0 + n],
                          in1=gview[:, :, s0:s0 + n], op=mybir.AluOpType.add)


def _build():
    if "nc" in _CACHE:
        return _CACHE["nc"]
    ops = _register_dve_ops()
    DKEY, SELMUL = ops["ANT_DKEY"], ops["ANT_SELMUL"]
    SELMAX = ops["ANT_SELMAX"]
    nc = bacc.Bacc("TRN2", target_bir_lowering=False, debug=False, num_devices=1)

    x_d = nc.dram_tensor("x", (BL, C_IN, H, W_IMG), F32, kind="ExternalInput").ap()
    w1_d = nc.dram_tensor("w1i", (27, HID), F32, kind="ExternalInput").ap()
    b1_d = nc.dram_tensor("b1", (128, 2), F32, kind="ExternalInput").ap()
    w2_d = nc.dram_tensor("w2e", (128, 2 * C), F32, kind="ExternalInput").ap()
    b2_d = nc.dram_tensor("b2", (1, C), F32, kind="ExternalInput").ap()
    startrep_d = nc.dram_tensor("start_rep", (BL, C), F32, kind="ExternalInput").ap()
    endrep_d = nc.dram_tensor("end_rep", (BL, C), F32, kind="ExternalInput").ap()
    trans1_d = nc.dram_tensor("trans1", (1, F), F32, kind="ExternalInput").ap()
    negi21b_d = nc.dram_tensor("negi21b", (BL, C), F32, kind="ExternalInput").ap()

    tags_d = nc.dram_tensor("tags", (BL, L), I8, kind="ExternalOutput").ap()

    em_d = nc.dram_tensor("em_hbm", (BL, EMLEN, C), F32, kind="Internal").ap()
    bounce_d = nc.dram_tensor("bounce", (BL, K * C), F32, kind="Internal").ap()
    gsum_d = nc.dram_tensor("gsum", (2, CH), F32, kind="Internal").ap()

    em_flat = [em_d[b].rearrange("t n -> (t n)") for b in range(BL)]

    with TileContext(nc) as tc:
        emtp = tc.tile_pool(name="emtp", bufs=1)
        ep = emtp.__enter__()
        emt = ep.tile([CH, ULEN * C], F32)
        # ====================== conv backbone ======================
        with tc.tile_pool(name="convpool", bufs=1) as cp, \
             tc.tile_pool(name="convwork", bufs=4) as cw, \
             tc.tile_pool(name="convpsum", bufs=3, space="PSUM") as cpp:
            im2c = [cp.tile([27, L], F32, name=f"im2c{b}") for b in range(BL)]
            w1sb = cp.tile([27, HID], F32)
            b1sb = cp.tile([128, 2], F32)
            w2sb = cp.tile([128, 2 * C], F32)
            b2sb = cp.tile([1, C], F32)
            onesb = cp.tile([1, 128], F32)
            zseg = cp.tile([1, EMPAD * C], F32)
            nc.gpsimd.memset(zseg[:], 0.0)
            for b in range(BL):
                nc.gpsimd.dma_start(
                    AP(tensor=em_flat[b].tensor, offset=b * EMLEN * C,
                       ap=[[0, 1], [1, EMPAD * C]]),
                    zseg[:])

            nc.sync.dma_start(w1sb[:], w1_d[:])
            nc.sync.dma_start(b1sb[:], b1_d[:])
            nc.sync.dma_start(w2sb[:], w2_d[:])
            nc.sync.dma_start(b2sb[:], b2_d[:])
            nc.vector.memset(onesb[:], 1.0)
            # once-only zero of the only cells no shift DMA ever writes
            # (y=0 for dy=0 rows 0-8, y=127 for dy=2 rows 18-26); they stay
            # zero across samples (SAME conv padding)
            for b in range(BL):
                for (r0, y0) in ((0, 0), (18, H - 1)):
                    nc.gpsimd.dma_start(
                        AP(tensor=im2c[b][:].tensor, offset=r0 * L + y0 * W_IMG,
                           ap=[[L, 9], [1, W_IMG]]),
                        AP(tensor=em_flat[0].tensor, offset=0,
                           ap=[[0, 9], [1, W_IMG]]))

            # im2col straight from DRAM x: per (dy, dx) a bulk DMA (plus a
            # single-row DMA where the column shift would read before the
            # channel plane); wrap-garbage border columns re-zeroed per sample
            dmaq = [nc.sync, nc.scalar, nc.gpsimd]
            HW = H * W_IMG

            def im2col_fill(b):
                # all top-half (y < 64) work first so the first matmul
                # blocks can start while the bottom halves stream in
                xb = b * C_IN * HW
                qi = [0]
                for half in (0, 1):
                    h0, h1 = (0, H // 2) if half == 0 else (H // 2, H)
                    for dy in range(3):
                        for dx in range(3):
                            r0 = (dy * 3 + dx) * 3
                            xoff = dx - 1
                            if dx == 0:
                                ylo = max(h0, 2 - dy)
                            else:
                                ylo = max(h0, 1 - dy)
                            if dx == 2:
                                yhi = min(h1, H - dy)    # keep r_s <= 126
                            else:
                                yhi = min(h1, H + 1 - dy)
                            nr = yhi - ylo
                            if nr > 0:
                                eng = dmaq[qi[0] % 3]; qi[0] += 1
                                eng.dma_start(
                                    AP(tensor=im2c[b][:].tensor,
                                       offset=r0 * L + ylo * W_IMG,
                                       ap=[[L, 3], [W_IMG, nr], [1, W_IMG]]),
                                    AP(tensor=x_d.tensor,
                                       offset=xb + (ylo + dy - 1) * W_IMG + xoff,
                                       ap=[[HW, 3], [W_IMG, nr], [1, W_IMG]]))
                            if half == 0 and dx == 0 and dy <= 1:
                                ys = 1 - dy       # src row 0, cols [0, 127)
                                eng = dmaq[qi[0] % 3]; qi[0] += 1
                                eng.dma_start(
                                    AP(tensor=im2c[b][:].tensor,
                                       offset=r0 * L + ys * W_IMG + 1,
                                       ap=[[L, 3], [1, W_IMG - 1]]),
                                    AP(tensor=x_d.tensor, offset=xb,
                                       ap=[[HW, 3], [1, W_IMG - 1]]))
                            if half == 1 and dx == 2 and dy >= 1:
                                ys = H - dy       # src row 127, cols [1, 128)
                                eng = dmaq[qi[0] % 3]; qi[0] += 1
                                eng.dma_start(
                                    AP(tensor=im2c[b][:].tensor,
                                       offset=r0 * L + ys * W_IMG,
                                       ap=[[L, 3], [1, W_IMG - 1]]),
                                    AP(tensor=x_d.tensor,
                                       offset=xb + (H - 1) * W_IMG + 1,
                                       ap=[[HW, 3], [1, W_IMG - 1]]))
                    # wrap-garbage border columns for this half (x=0 for
                    # dx=0, x=127 for dx=2); must follow the half's bulks
                    for dy in range(3):
                        for dx in (0, 2):
                            r0 = (dy * 3 + dx) * 3
                            col = 0 if dx == 0 else W_IMG - 1
                            eng = dmaq[qi[0] % 3]; qi[0] += 1
                            eng.dma_start(
                                AP(tensor=im2c[b][:].tensor,
                                   offset=r0 * L + h0 * W_IMG + col,
                                   ap=[[L, 3], [W_IMG, H // 2], [1, 1]]),
                                AP(tensor=em_flat[0].tensor, offset=0,
                                   ap=[[0, 3], [0, H // 2], [1, 1]]))

            for b in range(BL):
                im2col_fill(b)
                im2col = im2c[b]

                for tt in range(0, L, 512):
                    hid0 = cw.tile([128, 512], F32, tag="hid0")
                    hid1 = cw.tile([128, 512], F32, tag="hid1")
                    for hti, hid in ((0, hid0), (1, hid1)):
                        ps = cpp.tile([128, 512], F32, tag="psc1")
                        nc.tensor.matmul(
                            ps[:], w1sb[:, hti * 128:(hti + 1) * 128],
                            im2col[:, tt:tt + 512], start=True, stop=True)
                        nc.scalar.activation(
                            hid[:], ps[:], mybir.ActivationFunctionType.Relu,
                            bias=b1sb[:, hti:hti + 1], scale=1.0)
                    ps2 = cpp.tile([128, 4 * C], F32, tag="psc2")
                    for blk in range(4):
                        t0 = blk * 128
                        o = ps2[:, blk * C:(blk + 1) * C]
                        nc.tensor.matmul(o, hid0[:, t0:t0 + 128], w2sb[:, 0:C],
                                         start=True, stop=False)
                        nc.tensor.matmul(o, hid1[:, t0:t0 + 128], w2sb[:, C:2 * C],
                                         start=False, stop=False)
                        nc.tensor.matmul(o, onesb[:], b2sb[:], start=False, stop=True)
                    emst = cw.tile([128, 4 * C], F32, tag="emst")
                    nc.scalar.activation(
                        emst[:], ps2[:], mybir.ActivationFunctionType.Copy, scale=1.0)
                    nc.sync.dma_start(
                        em_d[b, EMPAD + tt:EMPAD + tt + 512]
                        .rearrange("(blk p) n -> p blk n", p=128),
                        emst[:].rearrange("p (blk n) -> p blk n", n=C),
                    )
                # em chunk rows: chain (b,c) covers t in [cS-WU+1, cS+S]
                nc.sync.dma_start(
                    emt[b * K:(b + 1) * K, :],
                    AP(tensor=em_flat[b].tensor, offset=b * EMLEN * C,
                       ap=[[S * C, K], [1, ULEN * C]]))

        # ====================== viterbi ======================
        with tc.tile_pool(name="vit", bufs=1) as vp:
            transrep = vp.tile([CH, F], F32)
            score = vp.tile([CH, C], F32)
            tmp_ring = vp.tile([CH, 16 * F], F32)
            scanring = vp.tile([CH, 16 * F], F32)
            maskbig = vp.tile([CH, F], F32)
            eqb = vp.tile([CH, 8 * F], F32)
            hist = vp.tile([CH, S * C], F32)
            paths = vp.tile([CH, S * C], F32)
            s0t = vp.tile([BL, C], F32)
            fs = vp.tile([BL, C], F32)
            startrep = vp.tile([BL, C], F32)
            endrep = vp.tile([BL, C], F32)
            negi21b = vp.tile([BL, C], F32)
            ltoh = vp.tile([BL, C], F32)
            ohc = vp.tile([BL, K * C], F32)
            fmap = vp.tile([BL, K * C], F32)
            selmask = vp.tile([CH, C], F32)
            ohw = vp.tile([BL, C], F32)
            small = vp.tile([BL, C], F32)
            small1 = vp.tile([BL, 1], F32)
            tagsf = vp.tile([CH, S], F32)
            tags8 = vp.tile([CH, S], I8)
            prod = vp.tile([CH, F], F32)
            msum0 = vp.tile([CH, 1], F32)
            msum1 = vp.tile([CH, 1], F32)
            gp = vp.tile([1, 3 * CH], F32)
            vinit = vp.tile([CH, 1], F32)

            nc.sync.dma_start(
                transrep[:],
                AP(tensor=trans1_d.tensor, offset=0, ap=[[0, CH], [1, F]]))
            nc.sync.dma_start(startrep[:], startrep_d[:])
            nc.sync.dma_start(endrep[:], endrep_d[:])
            nc.sync.dma_start(negi21b[:], negi21b_d[:])
            # maskbig: -BIG at the start of each 21-group (max-scan reset)
            nc.gpsimd.memset(maskbig[:], 0.0)
            nc.gpsimd.memset(
                maskbig[:].rearrange("p (g b) -> p g b", b=C)[:, :, 0:1], -BIG)

            # score0 = em[t=0] + start
            nc.sync.dma_start(s0t[:], em_d[:, EMPAD, :])
            nc.vector.tensor_tensor(out=s0t[:], in0=s0t[:], in1=startrep[:], op=AT.add)

            def emsl(u):
                return emt[:, u * C:(u + 1) * C]

            def step(u_em, slot):
                tmp_cur = tmp_ring[:, slot * F:(slot + 1) * F]
                scan_cur = scanring[:, slot * F:(slot + 1) * F]
                nc.vector.tensor_tensor(
                    out=_r3(tmp_cur), in0=score[:].unsqueeze(1).broadcast_to((CH, C, C)),
                    in1=_r3(transrep[:]), op=AT.add)
                # group-max over prev-tag b via reset-scan: out[:, a*C+C-1] = m[a]
                nc.vector.tensor_tensor_scan(
                    out=scan_cur, data0=maskbig[:, 0:F], data1=tmp_cur,
                    initial=0.0, op0=AT.add, op1=AT.max)
                m_view = _r3(scan_cur)[:, :, C - 1:C].rearrange("p a b -> p (a b)")
                nc.vector.tensor_tensor(out=score[:], in0=m_view, in1=emsl(u_em),
                                        op=AT.add)

            # ---------- pass 1 (magnitude gain probe) ----------
            nc.vector.memset(score[:], 0.0)
            for s in range(P1W + P1G):
                step(WU - P1W + s, 0)
                if s == P1W - 1:
                    nc.vector.tensor_reduce(out=msum0[:], in_=score[:], axis=AX.X, op=AT.add)
            nc.vector.tensor_reduce(out=msum1[:], in_=score[:], axis=AX.X, op=AT.add)

            nc.sync.dma_start(AP(tensor=gsum_d.tensor, offset=0, ap=[[1, CH], [1, 1]]), msum0[:])
            nc.sync.dma_start(AP(tensor=gsum_d.tensor, offset=CH, ap=[[1, CH], [1, 1]]), msum1[:])
            nc.sync.dma_start(gp[0:1, 0:2 * CH], gsum_d[:].rearrange("a b -> (a b)").unsqueeze(0))
            # g*S per chain -> gp[0, 2CH:3CH]
            nc.vector.tensor_tensor(out=gp[0:1, 2 * CH:3 * CH], in0=gp[0:1, CH:2 * CH],
                                    in1=gp[0:1, 0:CH], op=AT.subtract)
            nc.vector.tensor_scalar(out=gp[0:1, 2 * CH:3 * CH], in0=gp[0:1, 2 * CH:3 * CH],
                                    scalar1=float(S) / (C * P1G), scalar2=None, op0=AT.mult)
            # exclusive prefix into gp[0, 0:CH] (ping-pong to avoid in-place hazard)
            pfa = vp.tile([1, CH], F32)
            pfb = vp.tile([1, CH], F32)
            nc.vector.memset(pfa[:], 0.0)
            nc.vector.tensor_copy(pfa[0:1, 1:K], gp[0:1, 2 * CH:2 * CH + K - 1])
            nc.vector.tensor_copy(pfa[0:1, K + 1:2 * K], gp[0:1, 2 * CH + K:2 * CH + 2 * K - 1])
            cur, nxt = pfa, pfb
            for sh in (1, 2, 4, 8, 16, 32):
                for h0 in (0, K):
                    nc.vector.tensor_copy(nxt[0:1, h0:h0 + sh], cur[0:1, h0:h0 + sh])
                    nc.vector.tensor_tensor(
                        out=nxt[0:1, h0 + sh:h0 + K], in0=cur[0:1, h0 + sh:h0 + K],
                        in1=cur[0:1, h0:h0 + K - sh], op=AT.add)
                cur, nxt = nxt, cur
            nc.vector.tensor_copy(gp[0:1, 0:CH], cur[0:1, 0:CH])
            # per-sample base mean(score0)/C
            nc.vector.tensor_reduce(out=small1[:], in_=s0t[:], axis=AX.X, op=AT.add)
            nc.vector.tensor_scalar(out=small1[:], in0=small1[:], scalar1=1.0 / C,
                                    scalar2=None, op0=AT.mult)
            nc.sync.dma_start(bounce_d[0:1, 0:1], small1[0:1, :])
            nc.sync.dma_start(bounce_d[0:1, 1:2], small1[1:2, :])
            base2 = vp.tile([1, 2], F32)
            nc.sync.dma_start(base2[:], bounce_d[0:1, 0:2])
            nc.vector.tensor_scalar(out=gp[0:1, 0:K], in0=gp[0:1, 0:K],
                                    scalar1=base2[0:1, 0:1], scalar2=None, op0=AT.add)
            nc.vector.tensor_scalar(out=gp[0:1, K:2 * K], in0=gp[0:1, K:2 * K],
                                    scalar1=base2[0:1, 1:2], scalar2=None, op0=AT.add)
            nc.sync.dma_start(gsum_d[0:1, :], gp[0:1, 0:CH])
            nc.sync.dma_start(vinit[:], AP(tensor=gsum_d.tensor, offset=0, ap=[[1, CH], [1, 1]]))

            # ---------- pass 2 (forward) ----------
            nc.vector.memset(score[:], 0.0)
            nc.vector.tensor_scalar(out=score[:], in0=score[:], scalar1=vinit[:, :],
                                    scalar2=None, op0=AT.add)
            for s in range(WU + S):
                if s == WU:
                    # chunk 0 records from the exact t=0 state
                    nc.sync.dma_start(score[0:1, :], s0t[0:1, :])
                    nc.sync.dma_start(score[K:K + 1, :], s0t[1:2, :])
                rec = s >= WU
                r = s - WU
                slot = (r % 16) if rec else 15
                step(s, slot)
                if s == WU + S - 2:
                    nc.sync.dma_start(fs[0:1, :], score[K - 1:K, :])
                    nc.sync.dma_start(fs[1:2, :], score[CH - 1:CH, :])
                if rec and (r % 8 == 7):
                    r0 = r - 7
                    s0 = slot - 7          # 0 or 8: batch ring half
                    # running-max == final max from the first argmax on, so
                    # group-summing the equality gives C - b* (first-tie)
                    sview = scanring[:, s0 * F:(s0 + 8) * F] \
                        .rearrange("p (s a b) -> p s a b", a=C, b=C)
                    mview = sview[:, :, :, C - 1:C].broadcast_to((CH, 8, C, C))
                    eview = eqb[:].rearrange("p (s a b) -> p s a b", a=C, b=C)
                    nc.gpsimd.tensor_tensor(out=eview, in0=sview, in1=mview,
                                            op=AT.subtract)
                    nc.gpsimd.tensor_scalar(out=eqb[:], in0=eqb[:], scalar1=0.0,
                                            scalar2=None, op0=AT.is_equal)
                    gview = eqb[:].rearrange("p (g b) -> p g b", b=C)
                    _sum_tree(nc.gpsimd, gview)
                    hview = hist[:, r0 * C:(r0 + 8) * C] \
                        .rearrange("p (s a) -> p s a", a=C)
                    nc.gpsimd.tensor_scalar(
                        out=hview,
                        in0=eqb[:].rearrange("p (s a b) -> p s a b", a=C, b=C)
                        [:, :, :, 0:1].rearrange("p s a b -> p s (a b)"),
                        scalar1=-float(C), scalar2=None, op0=AT.add)

            # identity-fix hist row S-1 of last chain of each sample
            nc.sync.dma_start(hist[K - 1:K, (S - 1) * C:], negi21b_d[0:1, :])
            nc.sync.dma_start(hist[CH - 1:CH, (S - 1) * C:], negi21b_d[0:1, :])

            # last tag onehot
            nc.vector.tensor_tensor(out=fs[:], in0=fs[:], in1=endrep[:], op=AT.add)
            nc.vector.tensor_reduce(out=small1[:], in_=fs[:], axis=AX.X, op=AT.max)
            nc.vector.tensor_scalar(out=small[:], in0=fs[:], scalar1=small1[:, :],
                                    scalar2=BIG, op0=AT.subtract, op1=AT.mult)
            nc.vector.tensor_tensor(out=small[:], in0=small[:], in1=negi21b[:], op=AT.add)
            nc.vector.tensor_reduce(out=small1[:], in_=small[:], axis=AX.X, op=AT.max)
            nc.vector.tensor_scalar(out=ltoh[:], in0=small[:], scalar1=small1[:, :],
                                    scalar2=None, op0=AT.is_equal)

            # ---------- pass 3: hi/lo split backtrace, Pool sum-scans ----------
            idprev = vp.tile([CH, C], F32)
            nc.sync.dma_start(idprev[:],
                              AP(tensor=negi21b_d.tensor, offset=0, ap=[[0, CH], [1, C]]))
            NQ = 4
            SQ = S // NQ  # 64 rows per stream
            prodq = [vp.tile([CH, 2 * F], F32, name=f"prodq{q}") for q in range(NQ)]

            def p3_q(q, rr):
                # select-or(0) then group-sum via Pool add-tree (in place);
                # the next iteration reads the tree roots directly
                r = q * SQ + rr
                ring = prodq[q]
                slot = ring[:, (rr % 2) * F:((rr % 2) + 1) * F]
                pslot = ring[:, ((rr + 1) % 2) * F:(((rr + 1) % 2) + 1) * F]
                prv = (idprev[:].unsqueeze(2) if rr == SQ - 1
                       else _r3(pslot)[:, :, 0:1]).broadcast_to((CH, C, C))
                hrow = hist[:, r * C:(r + 1) * C]
                nc.vector._custom_dve(
                    SELMUL, out=_r3(slot), in0=prv,
                    in1=hrow.unsqueeze(1).broadcast_to((CH, C, C)), s0=float(C))
                _sum_tree(nc.gpsimd, _r3(slot))
                nc.sync.dma_start(
                    paths[:, r * C:(r + 1) * C],
                    _r3(slot)[:, :, 0:1])

            for rr in range(SQ - 1, -1, -1):
                for q in range(NQ - 1, -1, -1):
                    p3_q(q, rr)

            # stitch: compose quarter maps into the full-chunk start map
            # fmap_full[i] = M0[M1[M2[M3[i]]]], Mq = paths row q*SQ
            fmap_full = vp.tile([CH, C], F32)
            gcur = paths[:, (NQ - 1) * SQ * C:((NQ - 1) * SQ + 1) * C]
            for q in (2, 1, 0):
                nc.vector._custom_dve(
                    SELMUL, out=_r3(prod[:]),
                    in0=gcur.unsqueeze(2).broadcast_to((CH, C, C)),
                    in1=paths[:, q * SQ * C:(q * SQ + 1) * C]
                    .unsqueeze(1).broadcast_to((CH, C, C)), s0=float(C))
                nc.vector.tensor_reduce(
                    out=fmap_full[:], in_=_r3(prod[:]), axis=AX.X, op=AT.add)
                gcur = fmap_full[:]

            # ---------- threading ----------
            for b in range(BL):
                nc.sync.dma_start(
                    AP(tensor=bounce_d.tensor, offset=b * K * C, ap=[[C, K], [1, C]]),
                    fmap_full[b * K:(b + 1) * K, :])
            nc.sync.dma_start(fmap[:], bounce_d[:])
            nc.vector.tensor_copy(ohw[:], ltoh[:])
            for c in range(K - 1, -1, -1):
                nc.vector.tensor_copy(ohc[:, c * C:(c + 1) * C], ohw[:])
                if c > 0:
                    nc.vector.tensor_tensor(out=small[:], in0=ohw[:],
                                            in1=fmap[:, c * C:(c + 1) * C], op=AT.mult)
                    nc.vector.tensor_reduce(out=small1[:], in_=small[:], axis=AX.X, op=AT.add)
                    nc.vector.tensor_scalar(out=ohw[:], in0=negi21b[:],
                                            scalar1=small1[:, :], scalar2=None,
                                            op0=AT.is_equal)
            nc.sync.dma_start(bounce_d[:], ohc[:])
            for b in range(BL):
                nc.sync.dma_start(
                    selmask[b * K:(b + 1) * K, :],
                    AP(tensor=bounce_d.tensor, offset=b * K * C, ap=[[C, K], [1, C]]))

            # ---------- selection + output ----------
            # quarter q rows select by the tag at its top boundary
            tqn = vp.tile([CH, 1], F32)
            selw = vp.tile([CH, C], F32)
            qmask = [vp.tile([CH, C], F32, name=f"qmask{q}") for q in range(NQ - 1)]
            mcur = selmask[:]
            for q in (2, 1, 0):
                nc.vector.tensor_tensor(
                    out=selw[:], in0=mcur,
                    in1=paths[:, (q + 1) * SQ * C:((q + 1) * SQ + 1) * C],
                    op=AT.mult)
                nc.vector.tensor_reduce(out=tqn[:], in_=selw[:], axis=AX.X, op=AT.add)
                nc.vector.tensor_scalar(out=qmask[q][:], in0=idprev[:],
                                        scalar1=tqn[:, :], scalar2=None,
                                        op0=AT.is_equal)
                mcur = qmask[q][:]
            for q in range(NQ):
                mk = selmask[:] if q == NQ - 1 else qmask[q][:]
                nc.gpsimd.tensor_tensor(
                    out=paths[:, q * SQ * C:(q + 1) * SQ * C]
                    .rearrange("p (r e) -> p r e", e=C),
                    in0=paths[:, q * SQ * C:(q + 1) * SQ * C]
                    .rearrange("p (r e) -> p r e", e=C),
                    in1=mk.unsqueeze(1).broadcast_to((CH, SQ, C)), op=AT.mult)
            nc.vector.tensor_reduce(
                out=tagsf[:], in_=paths[:].rearrange("p (r e) -> p r e", e=C),
                axis=AX.X, op=AT.add)
            nc.vector.tensor_scalar(out=tagsf[:], in0=tagsf[:], scalar1=-1.0,
                                    scalar2=None, op0=AT.mult)
            nc.vector.tensor_copy(tags8[:], tagsf[:])
            for b in range(BL):
                nc.sync.dma_start(
                    tags_d[b].rearrange("(c r) -> c r", r=S),
                    tags8[b * K:(b + 1) * K, :])

        emtp.__exit__(None, None, None)

    nc.compile()
    _CACHE["nc"] = nc
    return nc


def _make_in_maps(x, conv1_w, conv1_b, conv2_w, conv2_b, start_trans, end_trans,
                  trans):
    x = np.asarray(x, np.float32)
    trans = np.asarray(trans, np.float32)
    trans1 = np.ascontiguousarray(trans.T).reshape(1, F)
    w1i = np.ascontiguousarray(
        np.asarray(conv1_w, np.float32).transpose(2, 3, 1, 0).reshape(27, HID))
    b1 = np.ascontiguousarray(np.asarray(conv1_b, np.float32).reshape(2, 128).T)
    w2e = np.ascontiguousarray(
        np.asarray(conv2_w, np.float32).reshape(C, HID).T
        .reshape(2, 128, C).transpose(1, 0, 2).reshape(128, 2 * C))
    b2 = np.asarray(conv2_b, np.float32).reshape(1, C)
    startrep = np.tile(np.asarray(start_trans, np.float32).reshape(1, C), (BL, 1))
    endrep = np.tile(np.asarray(end_trans, np.float32).reshape(1, C), (BL, 1))
    negi21b = np.tile(-np.arange(C, dtype=np.float32)[None, :], (BL, 1))

    in_maps = []
    for core in range(NCORES):
        in_maps.append({
            "x": np.ascontiguousarray(x[core * BL:(core + 1) * BL]),
            "w1i": w1i, "b1": b1, "w2e": w2e, "b2": b2,
            "start_rep": startrep, "end_rep": endrep,
            "trans1": trans1, "negi21b": negi21b,
        })
    return in_maps


def _postprocess_core(tags):
    return np.asarray(tags).astype(np.int32).reshape(-1, H, W_IMG)


def kernel(x, conv1_w, conv1_b, conv2_w, conv2_b, start_trans, end_trans, trans):
    nc = _build()
    in_maps = _make_in_maps(x, conv1_w, conv1_b, conv2_w, conv2_b,
                            start_trans, end_trans, trans)
    import os
    trace = bool(os.environ.get("BASS_TRACE_RUN"))
    res = bass_utils.run_bass_kernel_spmd(nc, in_maps, core_ids=list(range(NCORES)),
                                          trace=trace)
    global LAST_EXEC_NS
    LAST_EXEC_NS = res.exec_time_ns
    out = np.concatenate([r["tags"].astype(np.int32) for r in res.results], axis=0)
    return out.reshape(B, H, W_IMG)
